# revision 23
# baseline (speedup 1.0000x reference)
"""Two-layer GCN + edge scoring on 8 Trainium2 NeuronCores.

Sharding: nodes are greedily load-balanced (snake over in-degree) into
392 bins of 128 (49 blocks x 8 cores); aggregation edges partitioned by
destination bin; weights replicated; three device-side AllGathers move
the per-node tables (hs1, hs2, h2) between phases.

The GCN norm dinv[src]*dinv[dst] is separable: tables are pre-scaled
(hs = dinv * (x@W)) and block outputs post-scaled, so the scatter-add is
mask^T @ gathered_rows matmuls in PSUM with pure 0/1 masks. Masks are
generated on-chip (is_equal of a replicated iota against broadcast dst
slots), gathers are large batched non-transpose dma_gathers
(single_packet=False: >64-desc calls are illegal as one SDMA packet),
and edge scores are per-edge dot products via DVE mult + reduce.
"""
import os
import sys

for p in ("/opt/trn_rl_repo", "/opt/pypackages"):
    if p not in sys.path:
        sys.path.insert(0, p)

import numpy as np

import concourse.bacc as bacc
import concourse.bass as bass
import concourse.mybir as mybir
import concourse.tile as tile
from concourse import bass_utils, library_config
from concourse.masks import make_identity

FP16 = mybir.dt.float16
F32 = mybir.dt.float32
I16 = mybir.dt.int16
AF = mybir.ActivationFunctionType
OP = mybir.AluOpType
AX = mybir.AxisListType

NC_CORES = 8
D_IN = 512
D_HID = 256
GMAX = 4096      # max indices per dma_gather call (HW-validated)
SC_CALL = 2048   # edges per scoring window (= its single gather call)
GRP = 3          # aggregation blocks per gather group


def _wrap_idx(idx, width):
    """int16 gather-index layout: [128, n/16], idx[i] at [i%16, i//16],
    replicated across the 8 groups of 16 partitions."""
    n = len(idx)
    assert n % 16 == 0
    t = np.asarray(idx, np.int16).reshape(n // 16, 16).T  # [16, n/16]
    out = np.tile(t, (8, 1))
    assert out.shape == (128, n // 16)
    if width > n // 16:
        out = np.concatenate(
            [out, np.zeros((128, width - n // 16), np.int16)], axis=1)
    return out


def prep_host(x, edge_index, W1, b1, W2, b2, n_cores=NC_CORES):
    N, d_in = x.shape
    E = edge_index.shape[1]
    d_hid = W1.shape[1]
    nblk = (N // n_cores + 127) // 128
    n_loc = nblk * 128
    NP = n_loc * n_cores
    half = NP // 2
    nbins = n_cores * nblk
    assert half < 32768

    src0 = np.asarray(edge_index[0], np.int64)
    dst0 = np.asarray(edge_index[1], np.int64)

    deg = np.bincount(dst0, minlength=N).astype(np.int64) + 1  # + self loop
    dinv = (1.0 / np.sqrt(deg)).astype(np.float32)

    # ---- balanced node -> (bin, slot): snake over load-sorted nodes ----
    order = np.argsort(-deg, kind="stable")
    pos = np.arange(N)
    r_of, k_of = pos // nbins, pos % nbins
    bin_sorted = np.where(r_of % 2 == 0, k_of, nbins - 1 - k_of)
    bin_of = np.empty(N, np.int64)
    bin_of[order] = bin_sorted
    slot_of = np.empty(N, np.int64)
    slot_of[order] = r_of
    core_of = bin_of // nblk

    # ---- aggregation edges (incl. self loops) ----
    es = np.concatenate([src0, np.arange(N)])
    ed = np.concatenate([dst0, np.arange(N)])
    e_bin = bin_of[ed]
    e_lo = core_of[es] < (n_cores // 2)   # src row in lower table half

    lo_cnt = np.bincount(e_bin[e_lo], minlength=nbins)
    hi_cnt = np.bincount(e_bin[~e_lo], minlength=nbins)

    # rank bins within each core by lo count (desc) -> block index; this
    # makes per-rank maxima across cores tight (compile-time chunk counts)
    rank_of = np.empty(nbins, np.int64)
    for c in range(n_cores):
        ids = np.arange(c * nblk, (c + 1) * nblk)
        rank_of[ids[np.argsort(-lo_cnt[ids], kind="stable")]] = \
            np.arange(nblk)
    blk_of = rank_of[bin_of]
    row_of = core_of * n_loc + blk_of * 128 + slot_of   # node -> padded row

    lo_r = np.zeros((n_cores, nblk), np.int64)
    hi_r = np.zeros((n_cores, nblk), np.int64)
    for b in range(nbins):
        lo_r[b // nblk, rank_of[b]] = lo_cnt[b]
        hi_r[b // nblk, rank_of[b]] = hi_cnt[b]
    lo_ch = np.ceil(lo_r.max(0) / 128).astype(int)   # chunks per rank
    hi_ch = np.ceil(hi_r.max(0) / 128).astype(int)
    nch_seg_max = int(max(lo_ch.max(), hi_ch.max()))
    nch_max = int((lo_ch + hi_ch).max())

    # ---- column layout: per group [lo chunks of ranks][hi chunks] ----
    groups = [(g, min(g + GRP, nblk)) for g in range(0, nblk, GRP)]
    bloff = np.zeros(nblk, int)
    bhoff = np.zeros(nblk, int)
    ginfo = []   # per group: (col0, lo_cols, hi_cols)
    col = 0
    for (gb0, gb1) in groups:
        c0 = col
        for b in range(gb0, gb1):
            bloff[b] = col
            col += lo_ch[b]
        lo_cols = col - c0
        for b in range(gb0, gb1):
            bhoff[b] = col
            col += hi_ch[b]
        ginfo.append((c0, lo_cols, col - c0 - lo_cols))
    total_ch = col
    gch_max = max(lc + hc for (_c, lc, hc) in ginfo)

    # ---- fill gidx / dstl ----
    e_core = e_bin // nblk
    e_rank = rank_of[e_bin]
    key = (e_core * nblk + e_rank) * 2 + (~e_lo)
    ordk = np.lexsort((row_of[es], key))
    ks = key[ordk]
    srow_s = row_of[es][ordk]
    slot_s = slot_of[ed][ordk]
    seg_start = np.searchsorted(ks, np.arange(nbins * 2))
    seg_end = np.searchsorted(ks, np.arange(nbins * 2), side="right")

    gidx = np.zeros((n_cores, 128, total_ch * 8), np.int16)
    dstl = np.full((n_cores, 128, total_ch), -1.0, np.float16)
    for c in range(n_cores):
        for r in range(nblk):
            for h, (cap_ch, base) in enumerate(
                    ((lo_ch[r], bloff[r]), (hi_ch[r], bhoff[r]))):
                k = ((c * nblk + r) * 2 + h)
                s, e = seg_start[k], seg_end[k]
                cnt = e - s
                cap = cap_ch * 128
                assert cnt <= cap
                vals = np.zeros(cap, np.int64)
                vals[:cnt] = srow_s[s:e] - (half if h else 0)
                gidx[c, :, base * 8:(base + cap_ch) * 8] = \
                    _wrap_idx(vals, cap_ch * 8)
                i = np.arange(cnt)
                dstl[c, i % 128, base + i // 128] = slot_s[s:e]

    # ---- scoring: E edges, contiguous per-core slices, 4-way grouped ----
    e_per = E // n_cores
    sc_s = row_of[src0]
    sc_d = row_of[dst0]
    sgroups = [[None] * 4 for _ in range(n_cores)]
    for c in range(n_cores):
        sl = slice(c * e_per, (c + 1) * e_per)
        ss, dd = sc_s[sl], sc_d[sl]
        for g in range(4):
            m = ((ss >= half) * 2 + (dd >= half)) == g
            sgroups[c][g] = np.nonzero(m)[0]
    g_sz = [((max(len(sgroups[c][g]) for c in range(n_cores)) + SC_CALL - 1)
             // SC_CALL) * SC_CALL for g in range(4)]
    tot_sc = sum(g_sz)
    calls = []   # (group, call_size)
    for g in range(4):
        r = g_sz[g]
        while r > 0:
            s = min(SC_CALL, r)
            calls.append((g, s))
            r -= s
    # packed per-window index layout: [sidx cols | didx cols] per window
    sdidx = np.zeros((n_cores, 128, tot_sc // 8), np.int16)
    perm = np.full((n_cores, tot_sc), -1, np.int64)
    for c in range(n_cores):
        off = 0
        for g in range(4):
            idxs = sgroups[c][g]
            ss = sc_s[c * e_per + idxs] - (half if g >= 2 else 0)
            dd = sc_d[c * e_per + idxs] - (half if g % 2 == 1 else 0)
            sa = np.zeros(g_sz[g], np.int64)
            da = np.zeros(g_sz[g], np.int64)
            sa[:len(idxs)] = ss
            da[:len(idxs)] = dd
            perm[c, off:off + len(idxs)] = idxs
            p = 0
            while p < g_sz[g]:
                s = min(SC_CALL, g_sz[g] - p)
                w0 = (off + p) // 8
                sdidx[c, :, w0:w0 + s // 16] = \
                    _wrap_idx(sa[p:p + s], s // 16)
                sdidx[c, :, w0 + s // 16:w0 + s // 8] = \
                    _wrap_idx(da[p:p + s], s // 16)
                p += s
            off += g_sz[g]

    # ---- dense per-core tensors ----
    W1h = np.asarray(W1, np.float32).reshape(4, 128, d_hid).transpose(1, 0, 2)
    W2h = np.asarray(W2, np.float32).reshape(2, 128, d_hid).transpose(1, 0, 2)
    b1c = np.asarray(b1, np.float32).reshape(2, 128).T.copy()
    b2r = np.tile(np.asarray(b2, np.float32)[None, :], (128, 1))
    iota = np.tile(np.arange(128, dtype=np.float16),
                   (128, nch_seg_max, 1))   # [128, seg, 128]

    Xp = np.zeros((NP, d_in), np.float32)
    Xp[row_of] = np.asarray(x, np.float32)
    dv = np.zeros(NP, np.float32)
    dv[row_of] = dinv

    in_maps = []
    for c in range(n_cores):
        xs = Xp[c * n_loc:(c + 1) * n_loc]
        xt = np.ascontiguousarray(
            xs.T.reshape(4, 128, nblk, 128).transpose(1, 2, 0, 3))
        dcol = dv[c * n_loc:(c + 1) * n_loc].reshape(nblk, 128).T.copy()
        in_maps.append({
            "xt": xt.astype(np.float16),
            "w1": W1h.astype(np.float16),
            "w2": W2h.astype(np.float16),
            "b1c": b1c, "b2r": b2r,
            "dinv": dcol,
            "iota": iota,
            "gidx": gidx[c],
            "dstl": dstl[c],
            "sdidx": sdidx[c],
        })

    cfg = dict(n_cores=n_cores, N=N, E=E, d_in=d_in, d_hid=d_hid,
               nblk=nblk, n_loc=n_loc, NP=NP, half=half,
               lo_ch=[int(v) for v in lo_ch],
               hi_ch=[int(v) for v in hi_ch],
               groups=groups, ginfo=ginfo,
               bloff=[int(v) for v in bloff],
               bhoff=[int(v) for v in bhoff],
               total_ch=total_ch, gch_max=gch_max,
               nch_seg_max=nch_seg_max, nch_max=nch_max,
               g_sz=g_sz, tot_sc=tot_sc, calls=calls,
               e_per=e_per)
    meta = dict(perm=perm)
    return in_maps, cfg, meta


def build_nc(cfg, timing_mode=False):
    n_cores = cfg["n_cores"]
    d_hid = cfg["d_hid"]
    nblk, n_loc, NP = cfg["nblk"], cfg["n_loc"], cfg["NP"]
    half = cfg["half"]
    lo_ch, hi_ch = cfg["lo_ch"], cfg["hi_ch"]
    groups, ginfo = cfg["groups"], cfg["ginfo"]
    bloff, bhoff = cfg["bloff"], cfg["bhoff"]
    total_ch, gch_max = cfg["total_ch"], cfg["gch_max"]
    nch_seg_max, nch_max = cfg["nch_seg_max"], cfg["nch_max"]
    tot_sc, calls = cfg["tot_sc"], cfg["calls"]


    nc = bacc.Bacc("TRN2", target_bir_lowering=False, debug=False,
                   num_devices=1 if timing_mode else n_cores)

    rg = [list(range(n_cores))]

    def all_gather(shard, full):
        if timing_mode:
            # timing workalike: local copy stands in for the collective;
            # real AG cost added by the harness
            nc.sync.dma_start(full[0:shard.shape[0], :], shard[:])
        else:
            nc.gpsimd.collective_compute(
                "AllGather", OP.bypass, replica_groups=rg,
                ins=[shard.opt()], outs=[full.opt()])

    t_xt = nc.dram_tensor("xt", [128, nblk, 4, 128], FP16,
                          kind="ExternalInput").ap()
    t_w1 = nc.dram_tensor("w1", [128, 4, d_hid], FP16,
                          kind="ExternalInput").ap()
    t_w2 = nc.dram_tensor("w2", [128, 2, d_hid], FP16,
                          kind="ExternalInput").ap()
    t_b1c = nc.dram_tensor("b1c", [128, 2], F32, kind="ExternalInput").ap()
    t_b2r = nc.dram_tensor("b2r", [128, d_hid], F32,
                           kind="ExternalInput").ap()
    t_dinv = nc.dram_tensor("dinv", [128, nblk], F32,
                            kind="ExternalInput").ap()
    t_iota = nc.dram_tensor("iota", [128, nch_seg_max, 128], FP16,
                            kind="ExternalInput").ap()
    t_gidx = nc.dram_tensor("gidx", [128, total_ch * 8], I16,
                            kind="ExternalInput").ap()
    t_dstl = nc.dram_tensor("dstl", [128, total_ch], FP16,
                            kind="ExternalInput").ap()
    t_sdidx = nc.dram_tensor("sdidx", [128, tot_sc // 8], I16,
                             kind="ExternalInput").ap()
    t_out = nc.dram_tensor("scores", [1, tot_sc], F32,
                           kind="ExternalOutput").ap()

    with tile.TileContext(nc) as tc:
        with (
            tc.tile_pool(name="const", bufs=1) as cp,
            tc.tile_pool(name="sbuf", bufs=3) as sb,
            tc.tile_pool(name="mask", bufs=3) as mp,
            tc.tile_pool(name="gath", bufs=2) as gp,
            tc.tile_pool(name="scg", bufs=2) as scp,
            tc.tile_pool(name="psum", bufs=3, space="PSUM") as ps,
            tc.tile_pool(name="psum_t", bufs=2, space="PSUM") as pst,
            tc.tile_pool(name="dram", bufs=1, space="DRAM") as dr,
        ):
            nc.gpsimd.load_library(library_config.mlp)

            # ---- resident constants ----
            w1_sb = cp.tile([128, 4, d_hid], FP16)
            nc.sync.dma_start(w1_sb[:], t_w1)
            w2_sb = cp.tile([128, 2, d_hid], FP16)
            nc.sync.dma_start(w2_sb[:], t_w2)
            b1c_sb = cp.tile([128, 2], F32)
            nc.sync.dma_start(b1c_sb[:], t_b1c)
            b2r_sb = cp.tile([128, d_hid], F32)
            nc.sync.dma_start(b2r_sb[:], t_b2r)
            dinv_sb = cp.tile([128, nblk], F32)
            nc.sync.dma_start(dinv_sb[:], t_dinv)
            iota_sb = cp.tile([128, nch_seg_max, 128], FP16)
            nc.sync.dma_start(iota_sb[:], t_iota)
            gidx_sb = cp.tile([128, total_ch * 8], I16)
            nc.sync.dma_start(gidx_sb[:], t_gidx)
            dstl_sb = cp.tile([128, total_ch], FP16)
            nc.sync.dma_start(dstl_sb[:], t_dstl)
            ident = cp.tile([128, 128], FP16)
            make_identity(nc, ident[:])
            ones16 = cp.tile([128, 1], FP16)
            nc.vector.memset(ones16[:], 1.0)
            h1T = cp.tile([128, nblk, 2, 128], FP16)

            # ---- DRAM tables ----
            hs1_shard = dr.tile([n_loc, d_hid], FP16)
            hs1_full = dr.tile([NP, d_hid], FP16)
            hs2_shard = dr.tile([n_loc, d_hid], FP16)
            hs2_full = dr.tile([NP, d_hid], FP16)
            h2_shard = dr.tile([n_loc, d_hid], FP16)
            h2_full = dr.tile([NP, d_hid], FP16)

            # ---- P0: GEMM1 + dinv pre-scale -> hs1_shard ----
            for b in range(nblk):
                xt_b = sb.tile([128, 4, 128], FP16, tag="xtb")
                nc.sync.dma_start(xt_b[:], t_xt[:, b, :, :])
                g1 = ps.tile([128, d_hid], F32, tag="mm")
                for k in range(4):
                    nc.tensor.matmul(
                        g1[:], lhsT=xt_b[:, k, :],
                        rhs=w1_sb[:, k, :], start=(k == 0), stop=(k == 3))
                hs1_b = sb.tile([128, d_hid], FP16, tag="hsb")
                nc.scalar.activation(hs1_b[:], g1[:], AF.Copy,
                                     scale=dinv_sb[:, b:b + 1])
                nc.sync.dma_start(hs1_shard[128 * b:128 * (b + 1), :],
                                  hs1_b[:])

            all_gather(hs1_shard, hs1_full)

            # ---- aggregation machinery ----
            def agg_group(gi, table_full, consume):
                """Gather the group's rows, then per block: build masks,
                matmul-accumulate, and hand PSUM to `consume(b, o)`."""
                b0, b1 = groups[gi]
                col0, lo_cols, hi_cols = ginfo[gi]
                g_t = gp.tile([128, gch_max, d_hid], FP16, tag="gath")
                for (cols, cbase, tbeg, tend) in (
                        (lo_cols, col0, 0, half),
                        (hi_cols, col0 + lo_cols, half, NP)):
                    p = 0
                    while p < cols * 128:
                        q = min(GMAX, cols * 128 - p)
                        rel = cbase - col0 + p // 128
                        nc.gpsimd.dma_gather(
                            g_t[:, rel:rel + q // 128, :],
                            table_full[tbeg:tend, :],
                            gidx_sb[:, (cbase * 8 + p // 16):
                                    (cbase * 8 + (p + q) // 16)],
                            q, q, d_hid, single_packet=False)
                        p += q
                for b in range(b0, b1):
                    nch_b = lo_ch[b] + hi_ch[b]
                    m = mp.tile([128, nch_max, 128], FP16, tag="mask")
                    mi = 0
                    for (seg_ch, sbase) in ((lo_ch[b], bloff[b]),
                                            (hi_ch[b], bhoff[b])):
                        if seg_ch == 0:
                            continue
                        d_bc = dstl_sb[:, sbase:sbase + seg_ch] \
                            .unsqueeze(2).broadcast_to([128, seg_ch, 128])
                        nc.vector.tensor_tensor(
                            m[:, mi:mi + seg_ch, :],
                            iota_sb[:, :seg_ch, :], d_bc, OP.is_equal)
                        mi += seg_ch
                    o = ps.tile([128, d_hid], F32, tag="mm")
                    mi = 0
                    for (seg_ch, sbase) in ((lo_ch[b], bloff[b]),
                                            (hi_ch[b], bhoff[b])):
                        for c in range(seg_ch):
                            nc.tensor.matmul(
                                o[:], lhsT=m[:, mi + c, :],
                                rhs=g_t[:, sbase - col0 + c, :],
                                start=(mi + c == 0),
                                stop=(mi + c == nch_b - 1))
                        mi += seg_ch
                    consume(b, o)

            # ---- P2: layer-1 aggregation -> h1T (SBUF, transposed) ----
            def consume1(b, o):
                tmp = sb.tile([128, d_hid], FP16, tag="tmp")
                nc.scalar.activation(tmp[:], o[:], AF.Copy,
                                     scale=dinv_sb[:, b:b + 1])
                for h in range(2):
                    tp = pst.tile([128, 128], FP16, tag="tps")
                    nc.tensor.transpose(tp[:], tmp[:, 128 * h:128 * (h + 1)],
                                        ident[:])
                    nc.scalar.activation(h1T[:, b, h, :], tp[:], AF.Relu,
                                         bias=b1c_sb[:, h:h + 1])

            for gi in range(len(groups)):
                agg_group(gi, hs1_full, consume1)

            # ---- P3: GEMM2 + dinv pre-scale -> hs2_shard ----
            for b in range(nblk):
                g2 = ps.tile([128, d_hid], F32, tag="mm")
                for k in range(2):
                    nc.tensor.matmul(g2[:], lhsT=h1T[:, b, k, :],
                                     rhs=w2_sb[:, k, :],
                                     start=(k == 0), stop=(k == 1))
                hs2_b = sb.tile([128, d_hid], FP16, tag="hsb")
                nc.scalar.activation(hs2_b[:], g2[:], AF.Copy,
                                     scale=dinv_sb[:, b:b + 1])
                nc.sync.dma_start(hs2_shard[128 * b:128 * (b + 1), :],
                                  hs2_b[:])

            all_gather(hs2_shard, hs2_full)

            # ---- P5: layer-2 aggregation -> h2_shard ----
            def consume2(b, o):
                tmp2 = sb.tile([128, d_hid], F32, tag="tmp2")
                nc.scalar.activation(tmp2[:], o[:], AF.Copy,
                                     scale=dinv_sb[:, b:b + 1])
                h2_b = sb.tile([128, d_hid], FP16, tag="h2b")
                nc.vector.tensor_tensor(h2_b[:], tmp2[:], b2r_sb[:], OP.add)
                nc.sync.dma_start(h2_shard[128 * b:128 * (b + 1), :],
                                  h2_b[:])

            for gi in range(len(groups)):
                agg_group(gi, hs2_full, consume2)

            all_gather(h2_shard, h2_full)

            # ---- P7: edge scoring (transposed dual gather + PE reduce) ----
            SC_RED = 512    # edges per PSUM ones-matmul reduction
            off = 0
            for (grp, csz) in calls:
                s_half = half if grp >= 2 else 0
                d_half = half if grp % 2 == 1 else 0
                sd_b = sb.tile([128, SC_CALL // 8], I16, tag="sdidxb")
                nc.sync.dma_start(sd_b[:, :csz // 8],
                                  t_sdidx[:, off // 8:(off + csz) // 8])
                srcg = scp.tile([128, 2, csz], FP16, tag=f"sg{csz}")
                nc.gpsimd.dma_gather(
                    srcg[:], h2_full[s_half:s_half + half, :],
                    sd_b[:, :csz // 16], csz, csz, d_hid,
                    transpose=True, single_packet=False)
                dstg = scp.tile([128, 2, csz], FP16, tag=f"dg{csz}")
                nc.gpsimd.dma_gather(
                    dstg[:], h2_full[d_half:d_half + half, :],
                    sd_b[:, csz // 16:csz // 8], csz, csz, d_hid,
                    transpose=True, single_packet=False)
                prod = scp.tile([128, 2, csz], FP16, tag=f"pr{csz}")
                nc.vector.tensor_tensor(prod[:], srcg[:], dstg[:], OP.mult)
                for c0 in range(0, csz, SC_RED):
                    rsz = min(SC_RED, csz - c0)
                    sps = pst.tile([1, SC_RED], F32, tag="scps")
                    for h in range(2):
                        nc.tensor.matmul(sps[:, :rsz], lhsT=ones16[:],
                                         rhs=prod[:, h, c0:c0 + rsz],
                                         start=(h == 0), stop=(h == 1))
                    sig = sb.tile([1, SC_RED], F32, tag="sig")
                    nc.scalar.activation(sig[:, :rsz], sps[:, :rsz],
                                         AF.Sigmoid)
                    nc.sync.dma_start(
                        t_out[:, off + c0:off + c0 + rsz], sig[:, :rsz])
                off += csz

    nc.compile()
    return nc


def _run(in_maps, cfg, meta, trace=False):
    nc = build_nc(cfg)
    res = bass_utils.run_bass_kernel_spmd(
        nc, in_maps, core_ids=list(range(cfg["n_cores"])), trace=trace)
    perm = meta["perm"]
    E, e_per = cfg["E"], cfg["e_per"]
    out = np.zeros(E, np.float32)
    for c in range(cfg["n_cores"]):
        vec = np.asarray(res.results[c]["scores"], np.float32).reshape(-1)
        valid = perm[c] >= 0
        out[c * e_per + perm[c][valid]] = vec[valid]
    return out, res


def kernel(x, edge_index, W1, b1, W2, b2):
    in_maps, cfg, meta = prep_host(
        np.asarray(x), np.asarray(edge_index), np.asarray(W1),
        np.asarray(b1), np.asarray(W2), np.asarray(b2))
    out, _res = _run(in_maps, cfg, meta,
                     trace=bool(int(os.environ.get("KERNEL_TRACE", "0"))))
    return out


# revision 30
# speedup vs baseline: 1.1311x; 1.1311x over previous
"""Two-layer GCN + edge scoring on 8 Trainium2 NeuronCores.

Sharding: nodes are greedily load-balanced (snake over in-degree) into
392 bins of 128 (49 blocks x 8 cores); aggregation edges partitioned by
destination bin; weights replicated; three device-side AllGathers move
the per-node tables (hs1, hs2, h2) between phases.

The GCN norm dinv[src]*dinv[dst] is separable: tables are pre-scaled
(hs = dinv * (x@W)) and block outputs post-scaled, so the scatter-add is
mask^T @ gathered_rows matmuls in PSUM with pure 0/1 masks. Masks are
generated on-chip (is_equal of a replicated iota against broadcast dst
slots), gathers are large batched non-transpose dma_gathers
(single_packet=False: >64-desc calls are illegal as one SDMA packet),
and edge scores are per-edge dot products via DVE mult + reduce.
"""
import os
import sys

for p in ("/opt/trn_rl_repo", "/opt/pypackages"):
    if p not in sys.path:
        sys.path.insert(0, p)

import numpy as np

import concourse.bacc as bacc
import concourse.bass as bass
import concourse.mybir as mybir
import concourse.tile as tile
from concourse import bass_utils, library_config
from concourse.masks import make_identity

FP16 = mybir.dt.float16
F32 = mybir.dt.float32
I16 = mybir.dt.int16
AF = mybir.ActivationFunctionType
OP = mybir.AluOpType
AX = mybir.AxisListType

NC_CORES = 8
D_IN = 512
D_HID = 256
GMAX = 4096      # max indices per dma_gather call (HW-validated)
SC_CALL = 2048   # edges per scoring window (= its single gather call)
GRP = 3          # aggregation blocks per gather group


def _wrap_idx(idx, width):
    """int16 gather-index layout: [128, n/16], idx[i] at [i%16, i//16],
    replicated across the 8 groups of 16 partitions."""
    n = len(idx)
    assert n % 16 == 0
    t = np.asarray(idx, np.int16).reshape(n // 16, 16).T  # [16, n/16]
    out = np.tile(t, (8, 1))
    assert out.shape == (128, n // 16)
    if width > n // 16:
        out = np.concatenate(
            [out, np.zeros((128, width - n // 16), np.int16)], axis=1)
    return out


def prep_host(x, edge_index, W1, b1, W2, b2, n_cores=NC_CORES):
    N, d_in = x.shape
    E = edge_index.shape[1]
    d_hid = W1.shape[1]
    nblk = (N // n_cores + 127) // 128
    n_loc = nblk * 128
    NP = n_loc * n_cores
    half = NP // 2
    nbins = n_cores * nblk
    assert half < 32768

    src0 = np.asarray(edge_index[0], np.int64)
    dst0 = np.asarray(edge_index[1], np.int64)

    deg = np.bincount(dst0, minlength=N).astype(np.int64) + 1  # + self loop
    dinv = (1.0 / np.sqrt(deg)).astype(np.float32)

    # ---- balanced node -> (bin, slot): snake over load-sorted nodes ----
    order = np.argsort(-deg, kind="stable")
    pos = np.arange(N)
    r_of, k_of = pos // nbins, pos % nbins
    bin_sorted = np.where(r_of % 2 == 0, k_of, nbins - 1 - k_of)
    bin_of = np.empty(N, np.int64)
    bin_of[order] = bin_sorted
    slot_of = np.empty(N, np.int64)
    slot_of[order] = r_of
    core_of = bin_of // nblk

    # ---- aggregation edges (incl. self loops) ----
    es = np.concatenate([src0, np.arange(N)])
    ed = np.concatenate([dst0, np.arange(N)])
    e_bin = bin_of[ed]
    e_lo = core_of[es] < (n_cores // 2)   # src row in lower table half

    lo_cnt = np.bincount(e_bin[e_lo], minlength=nbins)
    hi_cnt = np.bincount(e_bin[~e_lo], minlength=nbins)

    # rank bins within each core by lo count (desc) -> block index; this
    # makes per-rank maxima across cores tight (compile-time chunk counts)
    rank_of = np.empty(nbins, np.int64)
    for c in range(n_cores):
        ids = np.arange(c * nblk, (c + 1) * nblk)
        rank_of[ids[np.argsort(-lo_cnt[ids], kind="stable")]] = \
            np.arange(nblk)
    blk_of = rank_of[bin_of]
    row_of = core_of * n_loc + blk_of * 128 + slot_of   # node -> padded row

    lo_r = np.zeros((n_cores, nblk), np.int64)
    hi_r = np.zeros((n_cores, nblk), np.int64)
    for b in range(nbins):
        lo_r[b // nblk, rank_of[b]] = lo_cnt[b]
        hi_r[b // nblk, rank_of[b]] = hi_cnt[b]
    lo_ch = np.ceil(lo_r.max(0) / 128).astype(int)   # chunks per rank
    hi_ch = np.ceil(hi_r.max(0) / 128).astype(int)
    nch_seg_max = int(max(lo_ch.max(), hi_ch.max()))
    nch_max = int((lo_ch + hi_ch).max())

    # ---- column layout: per group [lo chunks of ranks][hi chunks] ----
    groups = [(g, min(g + GRP, nblk)) for g in range(0, nblk, GRP)]
    bloff = np.zeros(nblk, int)
    bhoff = np.zeros(nblk, int)
    ginfo = []   # per group: (col0, lo_cols, hi_cols)
    col = 0
    for (gb0, gb1) in groups:
        c0 = col
        for b in range(gb0, gb1):
            bloff[b] = col
            col += lo_ch[b]
        lo_cols = col - c0
        for b in range(gb0, gb1):
            bhoff[b] = col
            col += hi_ch[b]
        ginfo.append((c0, lo_cols, col - c0 - lo_cols))
    total_ch = col
    gch_max = max(lc + hc for (_c, lc, hc) in ginfo)

    # ---- fill gidx / dstl ----
    e_core = e_bin // nblk
    e_rank = rank_of[e_bin]
    key = (e_core * nblk + e_rank) * 2 + (~e_lo)
    ordk = np.lexsort((row_of[es], key))
    ks = key[ordk]
    srow_s = row_of[es][ordk]
    slot_s = slot_of[ed][ordk]
    seg_start = np.searchsorted(ks, np.arange(nbins * 2))
    seg_end = np.searchsorted(ks, np.arange(nbins * 2), side="right")

    gidx = np.zeros((n_cores, 128, total_ch * 8), np.int16)
    dstl = np.full((n_cores, 128, total_ch), -1.0, np.float16)
    for c in range(n_cores):
        for r in range(nblk):
            for h, (cap_ch, base) in enumerate(
                    ((lo_ch[r], bloff[r]), (hi_ch[r], bhoff[r]))):
                k = ((c * nblk + r) * 2 + h)
                s, e = seg_start[k], seg_end[k]
                cnt = e - s
                cap = cap_ch * 128
                assert cnt <= cap
                vals = np.zeros(cap, np.int64)
                vals[:cnt] = srow_s[s:e] - (half if h else 0)
                gidx[c, :, base * 8:(base + cap_ch) * 8] = \
                    _wrap_idx(vals, cap_ch * 8)
                i = np.arange(cnt)
                dstl[c, i % 128, base + i // 128] = slot_s[s:e]

    # ---- scoring: E edges, contiguous per-core slices, 4-way grouped ----
    e_per = E // n_cores
    sc_s = row_of[src0]
    sc_d = row_of[dst0]
    sgroups = [[None] * 4 for _ in range(n_cores)]
    for c in range(n_cores):
        sl = slice(c * e_per, (c + 1) * e_per)
        ss, dd = sc_s[sl], sc_d[sl]
        for g in range(4):
            m = ((ss >= half) * 2 + (dd >= half)) == g
            sgroups[c][g] = np.nonzero(m)[0]
    g_sz = [((max(len(sgroups[c][g]) for c in range(n_cores)) + 127)
             // 128) * 128 for g in range(4)]
    tot_sc = sum(g_sz)
    calls = []   # (group, call_size)
    for g in range(4):
        r = g_sz[g]
        while r > 0:
            s = min(SC_CALL, r)
            calls.append((g, s))
            r -= s
    # packed per-window index layout: [sidx cols | didx cols] per window
    sdidx = np.zeros((n_cores, 128, tot_sc // 8), np.int16)
    perm = np.full((n_cores, tot_sc), -1, np.int64)
    for c in range(n_cores):
        off = 0
        for g in range(4):
            idxs = sgroups[c][g]
            ss = sc_s[c * e_per + idxs] - (half if g >= 2 else 0)
            dd = sc_d[c * e_per + idxs] - (half if g % 2 == 1 else 0)
            sa = np.zeros(g_sz[g], np.int64)
            da = np.zeros(g_sz[g], np.int64)
            sa[:len(idxs)] = ss
            da[:len(idxs)] = dd
            perm[c, off:off + len(idxs)] = idxs
            p = 0
            while p < g_sz[g]:
                s = min(SC_CALL, g_sz[g] - p)
                w0 = (off + p) // 8
                sdidx[c, :, w0:w0 + s // 16] = \
                    _wrap_idx(sa[p:p + s], s // 16)
                sdidx[c, :, w0 + s // 16:w0 + s // 8] = \
                    _wrap_idx(da[p:p + s], s // 16)
                p += s
            off += g_sz[g]

    # ---- dense per-core tensors ----
    W1h = np.asarray(W1, np.float32).reshape(4, 128, d_hid).transpose(1, 0, 2)
    W2h = np.asarray(W2, np.float32).reshape(2, 128, d_hid).transpose(1, 0, 2)
    b1c = np.asarray(b1, np.float32).reshape(2, 128).T.copy()
    b2r = np.tile(np.asarray(b2, np.float32)[None, :], (128, 1))
    iota = np.tile(np.arange(128, dtype=np.float16),
                   (128, nch_seg_max, 1))   # [128, seg, 128]

    Xp = np.zeros((NP, d_in), np.float32)
    Xp[row_of] = np.asarray(x, np.float32)
    dv = np.zeros(NP, np.float32)
    dv[row_of] = dinv

    in_maps = []
    for c in range(n_cores):
        xs = Xp[c * n_loc:(c + 1) * n_loc]
        xt = np.ascontiguousarray(
            xs.T.reshape(4, 128, nblk, 128).transpose(1, 2, 0, 3))
        dcol = dv[c * n_loc:(c + 1) * n_loc].reshape(nblk, 128).T.copy()
        in_maps.append({
            "xt": xt.astype(np.float16),
            "w1": W1h.astype(np.float16),
            "w2": W2h.astype(np.float16),
            "b1c": b1c, "b2r": b2r,
            "dinv": dcol,
            "iota": iota,
            "gidx": gidx[c],
            "dstl": dstl[c],
            "sdidx": sdidx[c],
        })

    cfg = dict(n_cores=n_cores, N=N, E=E, d_in=d_in, d_hid=d_hid,
               nblk=nblk, n_loc=n_loc, NP=NP, half=half,
               lo_ch=[int(v) for v in lo_ch],
               hi_ch=[int(v) for v in hi_ch],
               groups=groups, ginfo=ginfo,
               bloff=[int(v) for v in bloff],
               bhoff=[int(v) for v in bhoff],
               total_ch=total_ch, gch_max=gch_max,
               nch_seg_max=nch_seg_max, nch_max=nch_max,
               g_sz=g_sz, tot_sc=tot_sc, calls=calls,
               e_per=e_per)
    meta = dict(perm=perm)
    return in_maps, cfg, meta


def build_nc(cfg, timing_mode=False):
    n_cores = cfg["n_cores"]
    d_hid = cfg["d_hid"]
    nblk, n_loc, NP = cfg["nblk"], cfg["n_loc"], cfg["NP"]
    half = cfg["half"]
    lo_ch, hi_ch = cfg["lo_ch"], cfg["hi_ch"]
    groups, ginfo = cfg["groups"], cfg["ginfo"]
    bloff, bhoff = cfg["bloff"], cfg["bhoff"]
    total_ch, gch_max = cfg["total_ch"], cfg["gch_max"]
    nch_seg_max, nch_max = cfg["nch_seg_max"], cfg["nch_max"]
    tot_sc, calls = cfg["tot_sc"], cfg["calls"]


    nc = bacc.Bacc("TRN2", target_bir_lowering=False, debug=False,
                   num_devices=1 if timing_mode else n_cores)

    rg = [list(range(n_cores))]

    def all_gather(shard, full):
        if timing_mode:
            # timing workalike: local copy stands in for the collective;
            # real AG cost added by the harness
            nc.sync.dma_start(full[0:shard.shape[0], :], shard[:])
        else:
            nc.gpsimd.collective_compute(
                "AllGather", OP.bypass, replica_groups=rg,
                ins=[shard.opt()], outs=[full.opt()])

    t_xt = nc.dram_tensor("xt", [128, nblk, 4, 128], FP16,
                          kind="ExternalInput").ap()
    t_w1 = nc.dram_tensor("w1", [128, 4, d_hid], FP16,
                          kind="ExternalInput").ap()
    t_w2 = nc.dram_tensor("w2", [128, 2, d_hid], FP16,
                          kind="ExternalInput").ap()
    t_b1c = nc.dram_tensor("b1c", [128, 2], F32, kind="ExternalInput").ap()
    t_b2r = nc.dram_tensor("b2r", [128, d_hid], F32,
                           kind="ExternalInput").ap()
    t_dinv = nc.dram_tensor("dinv", [128, nblk], F32,
                            kind="ExternalInput").ap()
    t_iota = nc.dram_tensor("iota", [128, nch_seg_max, 128], FP16,
                            kind="ExternalInput").ap()
    t_gidx = nc.dram_tensor("gidx", [128, total_ch * 8], I16,
                            kind="ExternalInput").ap()
    t_dstl = nc.dram_tensor("dstl", [128, total_ch], FP16,
                            kind="ExternalInput").ap()
    t_sdidx = nc.dram_tensor("sdidx", [128, tot_sc // 8], I16,
                             kind="ExternalInput").ap()
    t_out = nc.dram_tensor("scores", [128, tot_sc // 128], F32,
                           kind="ExternalOutput").ap()

    with tile.TileContext(nc) as tc:
        with (
            tc.tile_pool(name="const", bufs=1) as cp,
            tc.tile_pool(name="sbuf", bufs=3) as sb,
            tc.tile_pool(name="mask", bufs=3) as mp,
            tc.tile_pool(name="gath", bufs=2) as gp,
            tc.tile_pool(name="scg", bufs=3) as scp,
            tc.tile_pool(name="psum", bufs=3, space="PSUM") as ps,
            tc.tile_pool(name="psum_t", bufs=2, space="PSUM") as pst,
            tc.tile_pool(name="dram", bufs=1, space="DRAM") as dr,
        ):
            nc.gpsimd.load_library(library_config.mlp)

            # ---- resident constants ----
            w1_sb = cp.tile([128, 4, d_hid], FP16)
            nc.sync.dma_start(w1_sb[:], t_w1)
            w2_sb = cp.tile([128, 2, d_hid], FP16)
            nc.sync.dma_start(w2_sb[:], t_w2)
            b1c_sb = cp.tile([128, 2], F32)
            nc.sync.dma_start(b1c_sb[:], t_b1c)
            b2r_sb = cp.tile([128, d_hid], F32)
            nc.sync.dma_start(b2r_sb[:], t_b2r)
            dinv_sb = cp.tile([128, nblk], F32)
            nc.sync.dma_start(dinv_sb[:], t_dinv)
            iota_sb = cp.tile([128, nch_seg_max, 128], FP16)
            nc.sync.dma_start(iota_sb[:], t_iota)
            gidx_sb = cp.tile([128, total_ch * 8], I16)
            nc.sync.dma_start(gidx_sb[:], t_gidx)
            dstl_sb = cp.tile([128, total_ch], FP16)
            nc.sync.dma_start(dstl_sb[:], t_dstl)
            ident = cp.tile([128, 128], FP16)
            make_identity(nc, ident[:])
            ones16 = cp.tile([128, 1], FP16)
            nc.vector.memset(ones16[:], 1.0)
            h1T = cp.tile([128, nblk, 2, 128], FP16)

            # ---- DRAM tables ----
            hs1_shard = dr.tile([n_loc, d_hid], FP16)
            hs1_full = dr.tile([NP, d_hid], FP16)
            hs2_shard = dr.tile([n_loc, d_hid], FP16)
            hs2_full = dr.tile([NP, d_hid], FP16)
            h2_shard = dr.tile([n_loc, d_hid], FP16)
            h2_full = dr.tile([NP, d_hid], FP16)

            # ---- P0: GEMM1 + dinv pre-scale -> hs1_shard ----
            QB = 4   # blocks per batched load/store
            for q0 in range(0, nblk, QB):
                qn = min(QB, nblk - q0)
                xt_b = sb.tile([128, QB, 4, 128], FP16, tag="xtb")
                nc.sync.dma_start(xt_b[:, :qn, :, :],
                                  t_xt[:, q0:q0 + qn, :, :])
                hs_t = sb.tile([128, QB, d_hid], FP16, tag="hsb")
                for j in range(qn):
                    b = q0 + j
                    g1 = ps.tile([128, d_hid], F32, tag="mm")
                    for k in range(4):
                        nc.tensor.matmul(
                            g1[:], lhsT=xt_b[:, j, k, :],
                            rhs=w1_sb[:, k, :], start=(k == 0), stop=(k == 3))
                    nc.scalar.activation(hs_t[:, j, :], g1[:], AF.Copy,
                                         scale=dinv_sb[:, b:b + 1])
                nc.sync.dma_start(
                    hs1_shard[128 * q0:128 * (q0 + qn), :]
                    .rearrange("(b p) f -> p b f", b=qn),
                    hs_t[:, :qn, :])

            all_gather(hs1_shard, hs1_full)

            # ---- aggregation machinery ----
            def agg_group(gi, table_full, consume):
                """Gather the group's rows, then per block: build masks,
                matmul-accumulate, and hand PSUM to `consume(b, o)`."""
                b0, b1 = groups[gi]
                col0, lo_cols, hi_cols = ginfo[gi]
                g_t = gp.tile([128, gch_max, d_hid], FP16, tag="gath")
                for (cols, cbase, tbeg, tend) in (
                        (lo_cols, col0, 0, half),
                        (hi_cols, col0 + lo_cols, half, NP)):
                    p = 0
                    while p < cols * 128:
                        q = min(GMAX, cols * 128 - p)
                        rel = cbase - col0 + p // 128
                        nc.gpsimd.dma_gather(
                            g_t[:, rel:rel + q // 128, :],
                            table_full[tbeg:tend, :],
                            gidx_sb[:, (cbase * 8 + p // 16):
                                    (cbase * 8 + (p + q) // 16)],
                            q, q, d_hid, single_packet=False)
                        p += q
                for b in range(b0, b1):
                    nch_b = lo_ch[b] + hi_ch[b]
                    m = mp.tile([128, nch_max, 128], FP16, tag="mask")
                    mi = 0
                    for (seg_ch, sbase) in ((lo_ch[b], bloff[b]),
                                            (hi_ch[b], bhoff[b])):
                        if seg_ch == 0:
                            continue
                        d_bc = dstl_sb[:, sbase:sbase + seg_ch] \
                            .unsqueeze(2).broadcast_to([128, seg_ch, 128])
                        nc.vector.tensor_tensor(
                            m[:, mi:mi + seg_ch, :],
                            iota_sb[:, :seg_ch, :], d_bc, OP.is_equal)
                        mi += seg_ch
                    o = ps.tile([128, d_hid], F32, tag="mm")
                    mi = 0
                    for (seg_ch, sbase) in ((lo_ch[b], bloff[b]),
                                            (hi_ch[b], bhoff[b])):
                        for c in range(seg_ch):
                            nc.tensor.matmul(
                                o[:], lhsT=m[:, mi + c, :],
                                rhs=g_t[:, sbase - col0 + c, :],
                                start=(mi + c == 0),
                                stop=(mi + c == nch_b - 1))
                        mi += seg_ch
                    consume(b, o)

            # ---- P2: layer-1 aggregation -> h1T (SBUF, transposed) ----
            def consume1(b, o):
                tmp = sb.tile([128, d_hid], FP16, tag="tmp")
                nc.scalar.activation(tmp[:], o[:], AF.Copy,
                                     scale=dinv_sb[:, b:b + 1])
                for h in range(2):
                    tp = pst.tile([128, 128], FP16, tag="tps")
                    nc.tensor.transpose(tp[:], tmp[:, 128 * h:128 * (h + 1)],
                                        ident[:])
                    nc.scalar.activation(h1T[:, b, h, :], tp[:], AF.Relu,
                                         bias=b1c_sb[:, h:h + 1])

            for gi in range(len(groups)):
                agg_group(gi, hs1_full, consume1)

            # ---- P3: GEMM2 + dinv pre-scale -> hs2_shard ----
            for q0 in range(0, nblk, QB):
                qn = min(QB, nblk - q0)
                hs_t = sb.tile([128, QB, d_hid], FP16, tag="hsb")
                for j in range(qn):
                    b = q0 + j
                    g2 = ps.tile([128, d_hid], F32, tag="mm")
                    for k in range(2):
                        nc.tensor.matmul(g2[:], lhsT=h1T[:, b, k, :],
                                         rhs=w2_sb[:, k, :],
                                         start=(k == 0), stop=(k == 1))
                    nc.scalar.activation(hs_t[:, j, :], g2[:], AF.Copy,
                                         scale=dinv_sb[:, b:b + 1])
                nc.sync.dma_start(
                    hs2_shard[128 * q0:128 * (q0 + qn), :]
                    .rearrange("(b p) f -> p b f", b=qn),
                    hs_t[:, :qn, :])

            all_gather(hs2_shard, hs2_full)

            # ---- P5: layer-2 aggregation -> h2_shard ----
            def consume2(b, o):
                tmp2 = sb.tile([128, d_hid], F32, tag="tmp2")
                nc.scalar.activation(tmp2[:], o[:], AF.Copy,
                                     scale=dinv_sb[:, b:b + 1])
                h2_b = sb.tile([128, d_hid], FP16, tag="h2b")
                nc.vector.tensor_tensor(h2_b[:], tmp2[:], b2r_sb[:], OP.add)
                nc.sync.dma_start(h2_shard[128 * b:128 * (b + 1), :],
                                  h2_b[:])

            for gi in range(len(groups)):
                agg_group(gi, hs2_full, consume2)

            all_gather(h2_shard, h2_full)

            # ---- P7: edge scoring (dual gather + DVE dot) ----
            SCW = SC_CALL // 128
            off = 0
            for (grp, csz) in calls:
                s_half = half if grp >= 2 else 0
                d_half = half if grp % 2 == 1 else 0
                cw = csz // 128
                sd_b = sb.tile([128, SC_CALL // 8], I16, tag="sdidxb")
                nc.sync.dma_start(sd_b[:, :csz // 8],
                                  t_sdidx[:, off // 8:(off + csz) // 8])
                srcg = scp.tile([128, SCW, d_hid], FP16, tag="sg")
                nc.gpsimd.dma_gather(
                    srcg[:, :cw, :], h2_full[s_half:s_half + half, :],
                    sd_b[:, :csz // 16], csz, csz, d_hid,
                    single_packet=False)
                dstg = scp.tile([128, SCW, d_hid], FP16, tag="dg")
                nc.gpsimd.dma_gather(
                    dstg[:, :cw, :], h2_full[d_half:d_half + half, :],
                    sd_b[:, csz // 16:csz // 8], csz, csz, d_hid,
                    single_packet=False)
                nc.vector.tensor_tensor(srcg[:, :cw, :], srcg[:, :cw, :],
                                        dstg[:, :cw, :], OP.mult)
                red = sb.tile([128, SCW], F32, tag="red")
                nc.vector.tensor_reduce(red[:, :cw], srcg[:, :cw, :],
                                        AX.X, OP.add)
                sig = sb.tile([128, SCW], F32, tag="sig")
                nc.scalar.activation(sig[:, :cw], red[:, :cw], AF.Sigmoid)
                nc.sync.dma_start(t_out[:, off // 128:(off + csz) // 128],
                                  sig[:, :cw])
                off += csz

    nc.compile()
    return nc


def _run(in_maps, cfg, meta, trace=False):
    nc = build_nc(cfg)
    res = bass_utils.run_bass_kernel_spmd(
        nc, in_maps, core_ids=list(range(cfg["n_cores"])), trace=trace)
    perm = meta["perm"]
    E, e_per = cfg["E"], cfg["e_per"]
    out = np.zeros(E, np.float32)
    for c in range(cfg["n_cores"]):
        sc = np.asarray(res.results[c]["scores"], np.float32)
        vec = sc.T.reshape(-1)   # edge j of this core at sc[j%128, j//128]
        valid = perm[c] >= 0
        out[c * e_per + perm[c][valid]] = vec[valid]
    return out, res


def kernel(x, edge_index, W1, b1, W2, b2):
    in_maps, cfg, meta = prep_host(
        np.asarray(x), np.asarray(edge_index), np.asarray(W1),
        np.asarray(b1), np.asarray(W2), np.asarray(b2))
    out, _res = _run(in_maps, cfg, meta,
                     trace=bool(int(os.environ.get("KERNEL_TRACE", "0"))))
    return out
